# revision 20
# baseline (speedup 1.0000x reference)
"""Bi-Real Net binary conv2d (3x3, pad 1, stride 1) for Trainium2, 8 NeuronCores.

Math (forward values of the reference):
    xb = sign(x)                      in {-1, 0, +1}
    scale[o] = mean_{i,kh,kw} |w[o,i,kh,kw]|
    wb = scale[o] * sign(w)
    y = conv2d_NCHW(xb, wb, pad=1)

v3 kernel strategy (one core = 4 images, data-parallel over batch):
    - HW model (measured): a matmul issues every ~(N + 8) cycles at 2.4 GHz
      regardless of DoubleRow (DR doubles K per pass, not the stream rate),
      and the per-matmul implicit weight load is fully pipelined (free).
      So tensor time ~ #matmuls * N -> minimize matmul count * N.
    - Per image: DMA [128, 28, 112] f32 pieces -> SBUF, ACT Sign -> plane P0
      of a zero-padded fp8 buffer [128, 2, 114, 128]; gpsimd copies the
      column-shifted plane P1[r,c] = P0[r,c+1].
    - Conv as 5 matmuls per 4-output-row chunk, all N=448 (4-dim rhs APs,
      no garbage columns):
        * DR pairs (0,kw)+(1,kw), pair step = row pitch      (3 matmuls)
        * DR pair  (2,0)+(2,1),  pair step = plane stride    (1 matmul)
        * single   (2,2)                                     (1 matmul)
      = ceil(9 taps / 2) = the minimum pass count for K=128 channels.
    - PSUM (8 banks in flight) evacuated on DVE: multiply by per-channel
      scale[o], write fp16 stage tiles; output DMA'd as fp16 (halves output
      HBM traffic; |y| = scale*int with rel quant err 2^-11 << the 2e-2
      tolerance) and upcast to f32 on the host.
    - Emission is software-pipelined: image n+1's loads/signs/P1-copies are
      emitted before image n's compute so gpsimd store triggers never
      head-of-line-block the next image's P1 copies.
"""

import sys

sys.path.insert(0, "/opt/trn_rl_repo")

import numpy as np

import concourse.bacc as bacc
import concourse.bass as bass
import concourse.mybir as mybir
import concourse.tile as tile
from concourse.bass_utils import run_bass_kernel_spmd
from concourse.masks import make_identity

N_CORES = 8
B, C, H, W = 32, 128, 112, 112
BL = B // N_CORES  # images per core
HP = H + 2  # padded height (114)

F32 = mybir.dt.float32
F16 = mybir.dt.float16
BF16 = mybir.dt.bfloat16
FP8 = mybir.dt.float8e4
DR = mybir.MatmulPerfMode.DoubleRow

RP = 128  # fp8 padded-row pitch (DoubleRow pair step, must be %16)
NROWS = HP  # 114 rows; max row read is h0+5 = 113 (bottom pad)
PLANE = NROWS * RP  # P0 -> P1 stride (14592, %16 == 0)
N_LOADROWS = 28
N_SIGNROWS = 14
N_STAGEROWS = 28  # 7 chunks per stage
N_CHUNKS = H // 4  # 28

VARIANT = "v3"


def build_nc(variant="v3"):
    nc = bacc.Bacc(
        "TRN2", target_bir_lowering=False, debug=False, num_devices=N_CORES
    )
    x = nc.declare_dram_parameter("x", [BL, C, H, W], F32, isOutput=False)
    w = nc.declare_dram_parameter("weight", [C, C, 3, 3], F32, isOutput=False)
    y = nc.declare_dram_parameter("y", [BL, C, H, W], F16, isOutput=True)

    with tile.TileContext(nc) as tc:
        with (
            tc.tile_pool(name="consts", bufs=1) as consts,
            tc.tile_pool(name="raw", bufs=1) as raw_pool,
            tc.tile_pool(name="xpad", bufs=1) as xpad_pool,
            tc.tile_pool(name="stage", bufs=1) as stage_pool,
        ):
            # wdr[i, kw, j, o]: j=0 -> tap (0,kw), j=1 -> tap (1,kw)
            wdr = consts.tile([C, 3, 2, C], FP8)
            # wp2[i, j, o]: j=0 -> tap (2,0), j=1 -> tap (2,1); w22: tap (2,2)
            wp2 = consts.tile([C, 2, C], FP8)
            w22 = consts.tile([C, C], FP8)
            scale = consts.tile([C, 1], F32)
            identity = consts.tile([C, C], BF16)
            wf = consts.tile([C, C, 3, 3], F32)
            wabs = consts.tile([C, C, 3, 3], F32)
            ssum = consts.tile([C, 1], F32)
            wsign = consts.tile([C, C, 3, 3], BF16)

            # wf first on the input queue: it gates the whole weight-prep
            # chain (wsign -> transposes -> wdr), which is longer than the
            # first-signs chain that the image loads gate
            nc.sync.dma_start(wf[:, :, :, :], w[:, :, :, :])
            # prefetch image 0; first loads are small so the first signs
            # (and matmuls) can start early
            # NOTE: the input DMA queue is descriptor-rate-bound while cold
            # (~25 descriptors/us; one descriptor per partition) — smaller
            # first loads make the first data LATER, so keep 28-row pieces.
            raws0 = []
            r0 = 0
            for li in range(H // N_LOADROWS):
                raw = raw_pool.tile(
                    [C, N_LOADROWS, W], F32, tag="raw", bufs=5, name="raw"
                )
                nc.sync.dma_start(
                    raw[:, :, :], x[0][:, r0 : r0 + N_LOADROWS, :]
                )
                raws0.append((raw, r0, N_LOADROWS))
                r0 += N_LOADROWS
            make_identity(nc, identity)

            # ---- padded activation planes (double-buffered across images) --
            def border_memsets(xp):
                nc.gpsimd.memset(xp[:, 0, 0, 0:114], 0.0)  # P0 top pad row
                nc.gpsimd.memset(xp[:, 0, 113, 0:114], 0.0)  # P0 bottom pad row
                nc.gpsimd.memset(xp[:, 0, 1:113, 0], 0.0)  # P0 left pad col
                nc.gpsimd.memset(xp[:, 0, 1:113, 113], 0.0)  # P0 right pad col
                nc.gpsimd.memset(xp[:, 1, 113, 0:112], 0.0)  # P1 bottom pad row

            xpads = []
            for k in range(2):
                xp = xpad_pool.tile(
                    [C, 2, NROWS, RP], FP8, tag=f"xpad{k}", name=f"xpad{k}"
                )
                xpads.append(xp)
            border_memsets(xpads[0])  # buf 1 deferred past image 0's section

            # image 0 piece 0 (P0 sign on ACT, P1 copy on DVE) is emitted
            # before the weight prep so ACT signs it as soon as data lands
            nc.scalar.sign(
                xpads[0][:, 0, 1 : 1 + N_SIGNROWS, 1 : 1 + W],
                raws0[0][0][:, :N_SIGNROWS, :],
            )
            nc.vector.tensor_copy(
                xpads[0][:, 1, 1 : 1 + N_SIGNROWS, 0:W],
                xpads[0][:, 0, 1 : 1 + N_SIGNROWS, 1 : 1 + W],
            )

            with tc.tile_pool(name="wpsum", bufs=1, space="PSUM") as wpsum:
                # warmup matmuls (identity @ identity): ramp the PE p-state
                # to the full 2.4 GHz clock while waiting for weights + first
                # signs (matmuls after an idle gap run at 1.2 GHz for ~3us)
                warm = wpsum.tile([C, C], F32, tag="warm", bufs=1, name="warm")
                for _ in range(40):
                    nc.tensor.matmul(warm[:, :], identity[:, :], identity[:, :],
                                     start=True, stop=True)
                nc.scalar.sign(wsign[:, :, :, :], wf[:, :, :, :])
                for kh in range(3):
                    for kw in range(3):
                        pst = wpsum.tile([C, C], BF16, tag="pst", bufs=2, name="pst")
                        nc.tensor.transpose(
                            pst[:, :], wsign[:, :, kh, kw], identity[:, :]
                        )
                        if kh < 2:
                            dst = wdr[:, kw, kh, :]
                        elif kw < 2:
                            dst = wp2[:, kw, :]
                        else:
                            dst = w22[:, :]
                        nc.vector.tensor_copy(dst, pst[:, :])
                # scale chain: needed only by the first evacuation
                nc.scalar.activation(
                    wabs[:, :, :, :],
                    wf[:, :, :, :],
                    mybir.ActivationFunctionType.Abs,
                    accum_out=ssum[:, :],
                )
                nc.scalar.mul(scale[:, :], ssum[:, :], 1.0 / (C * 9))

            def emit_piece(xpad, raw, r0, a, piece):
                rr = r0 + a + 1
                nc.scalar.sign(
                    xpad[:, 0, rr : rr + N_SIGNROWS, 1 : 1 + W],
                    raw[:, a : a + N_SIGNROWS, :],
                )
                # P1 fill: mostly ACT (2nd sign), 1/4 on DVE copies
                # (gpsimd copies measured ~4 cy/elem — too slow). piece 0
                # goes to DVE so image 0's first chunk is not ACT-gated.
                if piece % 4 == 0:
                    nc.vector.tensor_copy(
                        xpad[:, 1, rr : rr + N_SIGNROWS, 0:W],
                        xpad[:, 0, rr : rr + N_SIGNROWS, 1 : 1 + W],
                    )
                else:
                    nc.scalar.sign(
                        xpad[:, 1, rr : rr + N_SIGNROWS, 0:W],
                        raw[:, a : a + N_SIGNROWS, :],
                    )

            def emit_input(n, skip_pieces=0):
                """Loads + signs (P0) + P1 copies for image n."""
                xpad = xpads[n % 2]
                if n == 0:
                    loads = raws0
                else:
                    loads = []
                    r0 = 0
                    for li in range(H // N_LOADROWS):
                        raw = raw_pool.tile(
                            [C, N_LOADROWS, W], F32, tag="raw", bufs=5,
                            name="raw",
                        )
                        nc.sync.dma_start(
                            raw[:, :, :], x[n][:, r0 : r0 + N_LOADROWS, :]
                        )
                        loads.append((raw, r0, N_LOADROWS))
                        r0 += N_LOADROWS
                piece = 0
                for raw, r0, rows in loads:
                    for a in range(0, rows, N_SIGNROWS):
                        if piece >= skip_pieces:
                            emit_piece(xpad, raw, r0, a, piece)
                        piece += 1

            def emit_compute(n):
                xpad = xpads[n % 2]
                yim = y[n]
                stages = {}
                for g in range(N_CHUNKS):
                    h0 = g * 4
                    ps = psum_pool.tile([C, 4, W], F32, tag="ps", bufs=8, name="ps")
                    for t in range(3):  # DR pairs (0,kw)+(1,kw)
                        base = xpad[:, 0, h0, t]
                        rhs = bass.AP(
                            tensor=base.tensor,
                            offset=base.offset,
                            ap=[base.ap[0], [RP, 2], [RP, 4], [1, W]],
                        )
                        nc.tensor.matmul(
                            ps[:, :, :], wdr[:, t, :, :], rhs,
                            start=(t == 0), stop=False, perf_mode=DR,
                        )
                    # DR pair (2,0)+(2,1) across planes
                    base = xpad[:, 0, h0 + 2, 0]
                    rhs = bass.AP(
                        tensor=base.tensor,
                        offset=base.offset,
                        ap=[base.ap[0], [PLANE, 2], [RP, 4], [1, W]],
                    )
                    nc.tensor.matmul(
                        ps[:, :, :], wp2[:, :, :], rhs,
                        start=False, stop=False, perf_mode=DR,
                    )
                    # single tap (2,2)
                    base = xpad[:, 0, h0 + 2, 2]
                    rhs = bass.AP(
                        tensor=base.tensor,
                        offset=base.offset,
                        ap=[base.ap[0], [RP, 4], [1, W]],
                    )
                    nc.tensor.matmul(
                        ps[:, :, :], w22[:, :], rhs, start=False, stop=True,
                    )
                    # evacuate
                    s_idx = g // 7
                    jr = (g % 7) * 4
                    if g % 7 == 0:
                        stages[s_idx] = stage_pool.tile(
                            [C, N_STAGEROWS, W], F16, tag="stage", bufs=3,
                            name="stage",
                        )
                    nc.vector.tensor_scalar_mul(
                        stages[s_idx][:, jr : jr + 4, :], ps[:, :, :],
                        scale[:, :],
                    )
                    # tail: for the last image's last two stages, store the
                    # first 16 rows as soon as they are evacuated so the
                    # output queue is nearly drained when compute ends
                    early_split = n == BL - 1 and s_idx >= 2
                    if early_split and g % 7 == 3:
                        s0 = s_idx * N_STAGEROWS
                        nc.gpsimd.dma_start(
                            yim[:, s0 : s0 + 16, :], stages[s_idx][:, :16, :]
                        )
                    if g % 7 == 6:
                        s0 = s_idx * N_STAGEROWS
                        if early_split:
                            nc.gpsimd.dma_start(
                                yim[:, s0 + 16 : s0 + N_STAGEROWS, :],
                                stages[s_idx][:, 16:, :],
                            )
                        else:
                            nc.gpsimd.dma_start(
                                yim[:, s0 : s0 + N_STAGEROWS, :],
                                stages[s_idx][:, :, :],
                            )

            with tc.tile_pool(name="psum", bufs=1, space="PSUM") as psum_pool:
                emit_input(0, skip_pieces=1)
                border_memsets(xpads[1])
                for n in range(BL):
                    if n + 1 < BL:
                        emit_input(n + 1)
                    emit_compute(n)

    nc.compile()
    return nc


_NC_CACHE = {}


def _get_nc(variant=None):
    variant = variant or VARIANT
    if variant not in _NC_CACHE:
        _NC_CACHE[variant] = build_nc(variant)
    return _NC_CACHE[variant]


def kernel(
    x: np.ndarray,
    weight: np.ndarray,
    _trace: bool = False,
    _variant: str | None = None,
    **_kw,
):
    assert x.shape == (B, C, H, W) and weight.shape == (C, C, 3, 3)
    nc = _get_nc(_variant)
    xs = np.ascontiguousarray(x, dtype=np.float32)
    wgt = np.ascontiguousarray(weight, dtype=np.float32)
    in_maps = [
        {"x": xs[i * BL : (i + 1) * BL], "weight": wgt} for i in range(N_CORES)
    ]
    res = run_bass_kernel_spmd(
        nc, in_maps, core_ids=list(range(N_CORES)), trace=_trace
    )
    out = np.concatenate(
        [np.asarray(res.results[i]["y"]) for i in range(N_CORES)], axis=0
    ).astype(np.float32)
    if _trace:
        kernel.last_results = res
    return out


# revision 21
# speedup vs baseline: 1.1831x; 1.1831x over previous
"""Bi-Real Net binary conv2d (3x3, pad 1, stride 1) for Trainium2, 8 NeuronCores.

Math (forward values of the reference):
    xb = sign(x)                      in {-1, 0, +1}
    scale[o] = mean_{i,kh,kw} |w[o,i,kh,kw]|
    wb = scale[o] * sign(w)
    y = conv2d_NCHW(xb, wb, pad=1)

v3 kernel strategy (one core = 4 images, data-parallel over batch):
    - HW model (measured): a matmul issues every ~(N + 8) cycles at 2.4 GHz
      regardless of DoubleRow (DR doubles K per pass, not the stream rate),
      and the per-matmul implicit weight load is fully pipelined (free).
      So tensor time ~ #matmuls * N -> minimize matmul count * N.
    - Per image: DMA [128, 28, 112] f32 pieces -> SBUF, ACT Sign -> plane P0
      of a zero-padded fp8 buffer [128, 2, 114, 128]; gpsimd copies the
      column-shifted plane P1[r,c] = P0[r,c+1].
    - Conv as 5 matmuls per 4-output-row chunk, all N=448 (4-dim rhs APs,
      no garbage columns):
        * DR pairs (0,kw)+(1,kw), pair step = row pitch      (3 matmuls)
        * DR pair  (2,0)+(2,1),  pair step = plane stride    (1 matmul)
        * single   (2,2)                                     (1 matmul)
      = ceil(9 taps / 2) = the minimum pass count for K=128 channels.
    - PSUM (8 banks in flight) evacuated on DVE: multiply by per-channel
      scale[o], write fp16 stage tiles; output DMA'd as fp16 (halves output
      HBM traffic; |y| = scale*int with rel quant err 2^-11 << the 2e-2
      tolerance) and upcast to f32 on the host.
    - Emission is software-pipelined: image n+1's loads/signs/P1-copies are
      emitted before image n's compute so gpsimd store triggers never
      head-of-line-block the next image's P1 copies.
"""

import sys

sys.path.insert(0, "/opt/trn_rl_repo")

import numpy as np

import concourse.bacc as bacc
import concourse.bass as bass
import concourse.mybir as mybir
import concourse.tile as tile
from concourse.bass_utils import run_bass_kernel_spmd
from concourse.masks import make_identity

N_CORES = 8
B, C, H, W = 32, 128, 112, 112
BL = B // N_CORES  # images per core
HP = H + 2  # padded height (114)

F32 = mybir.dt.float32
F16 = mybir.dt.float16
BF16 = mybir.dt.bfloat16
FP8 = mybir.dt.float8e4
DR = mybir.MatmulPerfMode.DoubleRow

RP = 128  # fp8 padded-row pitch (DoubleRow pair step, must be %16)
NROWS = HP  # 114 rows; max row read is h0+5 = 113 (bottom pad)
PLANE = NROWS * RP  # P0 -> P1 stride (14592, %16 == 0)
N_LOADROWS = 28
N_SIGNROWS = 14
N_STAGEROWS = 28  # 7 chunks per stage
N_CHUNKS = H // 4  # 28

VARIANT = "v3"


def build_nc(variant="v3"):
    nc = bacc.Bacc(
        "TRN2", target_bir_lowering=False, debug=False, num_devices=N_CORES
    )
    x = nc.declare_dram_parameter("x", [BL, C, H, W], F32, isOutput=False)
    w = nc.declare_dram_parameter("weight", [C, C, 3, 3], F32, isOutput=False)
    y = nc.declare_dram_parameter("y", [BL, C, H, W], F16, isOutput=True)

    with tile.TileContext(nc) as tc:
        with (
            tc.tile_pool(name="consts", bufs=1) as consts,
            tc.tile_pool(name="raw", bufs=1) as raw_pool,
            tc.tile_pool(name="xpad", bufs=1) as xpad_pool,
            tc.tile_pool(name="stage", bufs=1) as stage_pool,
        ):
            # wdr[i, kw, j, o]: j=0 -> tap (0,kw), j=1 -> tap (1,kw)
            wdr = consts.tile([C, 3, 2, C], FP8)
            # wp2[i, j, o]: j=0 -> tap (2,0), j=1 -> tap (2,1); w22: tap (2,2)
            wp2 = consts.tile([C, 2, C], FP8)
            w22 = consts.tile([C, C], FP8)
            scale = consts.tile([C, 1], F32)
            identity = consts.tile([C, C], BF16)
            wf = consts.tile([C, C, 3, 3], F32)
            wabs = consts.tile([C, C, 3, 3], F32)
            ssum = consts.tile([C, 1], F32)
            wsign = consts.tile([C, C, 3, 3], BF16)

            # wf first on the input queue: it gates the whole weight-prep
            # chain (wsign -> transposes -> wdr), which is longer than the
            # first-signs chain that the image loads gate
            nc.sync.dma_start(wf[:, :, :, :], w[:, :, :, :])
            # prefetch image 0; first loads are small so the first signs
            # (and matmuls) can start early
            # NOTE: the input DMA queue is descriptor-rate-bound while cold
            # (~25 descriptors/us; one descriptor per partition) — smaller
            # first loads make the first data LATER, so keep 28-row pieces.
            raws0 = []
            r0 = 0
            for li in range(H // N_LOADROWS):
                raw = raw_pool.tile(
                    [C, N_LOADROWS, W], F32, tag="raw", bufs=5, name="raw"
                )
                nc.sync.dma_start(
                    raw[:, :, :], x[0][:, r0 : r0 + N_LOADROWS, :]
                )
                raws0.append((raw, r0, N_LOADROWS))
                r0 += N_LOADROWS
            make_identity(nc, identity)

            # ---- padded activation planes (double-buffered across images) --
            def border_memsets(xp):
                nc.gpsimd.memset(xp[:, 0, 0, 0:114], 0.0)  # P0 top pad row
                nc.gpsimd.memset(xp[:, 0, 113, 0:114], 0.0)  # P0 bottom pad row
                nc.gpsimd.memset(xp[:, 0, 1:113, 0], 0.0)  # P0 left pad col
                nc.gpsimd.memset(xp[:, 0, 1:113, 113], 0.0)  # P0 right pad col
                nc.gpsimd.memset(xp[:, 1, 113, 0:112], 0.0)  # P1 bottom pad row

            xpads = []
            for k in range(2):
                xp = xpad_pool.tile(
                    [C, 2, NROWS, RP], FP8, tag=f"xpad{k}", name=f"xpad{k}"
                )
                xpads.append(xp)
            border_memsets(xpads[0])  # buf 1 deferred past image 0's section

            # image 0 piece 0 (P0 sign on ACT, P1 copy on DVE) is emitted
            # before the weight prep so ACT signs it as soon as data lands
            nc.scalar.sign(
                xpads[0][:, 0, 1 : 1 + N_SIGNROWS, 1 : 1 + W],
                raws0[0][0][:, :N_SIGNROWS, :],
            )
            nc.vector.tensor_copy(
                xpads[0][:, 1, 1 : 1 + N_SIGNROWS, 0:W],
                xpads[0][:, 0, 1 : 1 + N_SIGNROWS, 1 : 1 + W],
            )

            with tc.tile_pool(name="wpsum", bufs=1, space="PSUM") as wpsum:
                # warmup matmuls (identity @ identity): ramp the PE p-state
                # to the full 2.4 GHz clock while waiting for weights + first
                # signs (matmuls after an idle gap run at 1.2 GHz for ~3us)
                warm = wpsum.tile([C, C], F32, tag="warm", bufs=1, name="warm")
                for _ in range(40):
                    nc.tensor.matmul(warm[:, :], identity[:, :], identity[:, :],
                                     start=True, stop=True)
                nc.scalar.sign(wsign[:, :, :, :], wf[:, :, :, :])
                for kh in range(3):
                    for kw in range(3):
                        pst = wpsum.tile([C, C], BF16, tag="pst", bufs=2, name="pst")
                        nc.tensor.transpose(
                            pst[:, :], wsign[:, :, kh, kw], identity[:, :]
                        )
                        if kh < 2:
                            dst = wdr[:, kw, kh, :]
                        elif kw < 2:
                            dst = wp2[:, kw, :]
                        else:
                            dst = w22[:, :]
                        nc.vector.tensor_copy(dst, pst[:, :])
                # scale chain: needed only by the first evacuation
                nc.scalar.activation(
                    wabs[:, :, :, :],
                    wf[:, :, :, :],
                    mybir.ActivationFunctionType.Abs,
                    accum_out=ssum[:, :],
                )
                nc.scalar.mul(scale[:, :], ssum[:, :], 1.0 / (C * 9))

            def emit_piece(xpad, raw, r0, a, piece):
                rr = r0 + a + 1
                nc.scalar.sign(
                    xpad[:, 0, rr : rr + N_SIGNROWS, 1 : 1 + W],
                    raw[:, a : a + N_SIGNROWS, :],
                )
                # P1 fill: mostly ACT (2nd sign), 1/4 on DVE copies
                # (gpsimd copies measured ~4 cy/elem — too slow). piece 0
                # goes to DVE so image 0's first chunk is not ACT-gated.
                if piece % 4 == 0:
                    nc.vector.tensor_copy(
                        xpad[:, 1, rr : rr + N_SIGNROWS, 0:W],
                        xpad[:, 0, rr : rr + N_SIGNROWS, 1 : 1 + W],
                    )
                else:
                    nc.scalar.sign(
                        xpad[:, 1, rr : rr + N_SIGNROWS, 0:W],
                        raw[:, a : a + N_SIGNROWS, :],
                    )

            def emit_input(n, skip_pieces=0):
                """Loads + signs (P0) + P1 copies for image n."""
                xpad = xpads[n % 2]
                if n == 0:
                    loads = raws0
                else:
                    loads = []
                    r0 = 0
                    for li in range(H // N_LOADROWS):
                        raw = raw_pool.tile(
                            [C, N_LOADROWS, W], F32, tag="raw", bufs=5,
                            name="raw",
                        )
                        nc.sync.dma_start(
                            raw[:, :, :], x[n][:, r0 : r0 + N_LOADROWS, :]
                        )
                        loads.append((raw, r0, N_LOADROWS))
                        r0 += N_LOADROWS
                piece = 0
                for raw, r0, rows in loads:
                    for a in range(0, rows, N_SIGNROWS):
                        if piece >= skip_pieces:
                            emit_piece(xpad, raw, r0, a, piece)
                        piece += 1

            def emit_compute(n):
                xpad = xpads[n % 2]
                yim = y[n]
                stages = {}
                for g in range(N_CHUNKS):
                    h0 = g * 4
                    ps = psum_pool.tile([C, 4, W], F32, tag="ps", bufs=8, name="ps")
                    for t in range(3):  # DR pairs (0,kw)+(1,kw)
                        base = xpad[:, 0, h0, t]
                        rhs = bass.AP(
                            tensor=base.tensor,
                            offset=base.offset,
                            ap=[base.ap[0], [RP, 2], [RP, 4], [1, W]],
                        )
                        nc.tensor.matmul(
                            ps[:, :, :], wdr[:, t, :, :], rhs,
                            start=(t == 0), stop=False, perf_mode=DR,
                        )
                    # DR pair (2,0)+(2,1) across planes
                    base = xpad[:, 0, h0 + 2, 0]
                    rhs = bass.AP(
                        tensor=base.tensor,
                        offset=base.offset,
                        ap=[base.ap[0], [PLANE, 2], [RP, 4], [1, W]],
                    )
                    nc.tensor.matmul(
                        ps[:, :, :], wp2[:, :, :], rhs,
                        start=False, stop=False, perf_mode=DR,
                    )
                    # single tap (2,2)
                    base = xpad[:, 0, h0 + 2, 2]
                    rhs = bass.AP(
                        tensor=base.tensor,
                        offset=base.offset,
                        ap=[base.ap[0], [RP, 4], [1, W]],
                    )
                    nc.tensor.matmul(
                        ps[:, :, :], w22[:, :], rhs, start=False, stop=True,
                    )
                    # evacuate
                    s_idx = g // 7
                    jr = (g % 7) * 4
                    if g % 7 == 0:
                        stages[s_idx] = stage_pool.tile(
                            [C, N_STAGEROWS, W], F16, tag="stage", bufs=3,
                            name="stage",
                        )
                    nc.vector.tensor_scalar_mul(
                        stages[s_idx][:, jr : jr + 4, :], ps[:, :, :],
                        scale[:, :],
                    )
                    # tail: store the last image's stages in pieces as soon
                    # as rows are evacuated so the output queue is nearly
                    # drained when compute ends
                    s0 = s_idx * N_STAGEROWS
                    if n == BL - 1 and s_idx >= 2:
                        cuts = {3: (0, 16), 5: (16, 24), 6: (24, 28)}
                        if g % 7 in cuts:
                            a, b = cuts[g % 7]
                            nc.gpsimd.dma_start(
                                yim[:, s0 + a : s0 + b, :],
                                stages[s_idx][:, a:b, :],
                            )
                    elif g % 7 == 6:
                        nc.gpsimd.dma_start(
                            yim[:, s0 : s0 + N_STAGEROWS, :],
                            stages[s_idx][:, :, :],
                        )

            with tc.tile_pool(name="psum", bufs=1, space="PSUM") as psum_pool:
                emit_input(0, skip_pieces=1)
                border_memsets(xpads[1])
                for n in range(BL):
                    if n + 1 < BL:
                        emit_input(n + 1)
                    emit_compute(n)

    nc.compile()
    return nc


_NC_CACHE = {}


def _get_nc(variant=None):
    variant = variant or VARIANT
    if variant not in _NC_CACHE:
        _NC_CACHE[variant] = build_nc(variant)
    return _NC_CACHE[variant]


def kernel(
    x: np.ndarray,
    weight: np.ndarray,
    _trace: bool = False,
    _variant: str | None = None,
    **_kw,
):
    assert x.shape == (B, C, H, W) and weight.shape == (C, C, 3, 3)
    nc = _get_nc(_variant)
    xs = np.ascontiguousarray(x, dtype=np.float32)
    wgt = np.ascontiguousarray(weight, dtype=np.float32)
    in_maps = [
        {"x": xs[i * BL : (i + 1) * BL], "weight": wgt} for i in range(N_CORES)
    ]
    res = run_bass_kernel_spmd(
        nc, in_maps, core_ids=list(range(N_CORES)), trace=_trace
    )
    out = np.concatenate(
        [np.asarray(res.results[i]["y"]) for i in range(N_CORES)], axis=0
    ).astype(np.float32)
    if _trace:
        kernel.last_results = res
    return out
